# revision 91
# baseline (speedup 1.0000x reference)
"""GPT-2-style causal attention block on 8 TRN2 NeuronCores (Bass/Tile).

Sharding (Megatron-style, per the hint): core c handles batch b = c // 4 and
head-group g = c % 4 (4 of the 16 heads).  Each core computes, fully locally:
  QKV projection (its 4 heads' columns), causal softmax attention for its
  4 heads, and the row-sharded output projection partial [S, D].
The host gathers by summing the 4 partials per batch and adding c_proj_b.

Per-core kernel design (cost model: matmul time = moving-operand columns
only; start_tensor_calc zeroes a whole 2KB PSUM bank; ACT exp 0.83ns/col):
  - x^T [D, S] is staged on host so Q^T/K^T come out of matmuls with
    head_dim on partitions and V comes out as [seq, head_dim].
  - Phase A computes pair-0 Q/K k-OUTER across 8 live PSUM accumulators, so
    each arriving x^T k-tile immediately feeds 8x512 cols of PE work and the
    PE tracks the input DMA stream.
  - Scores are computed transposed (key index on partitions), two heads per
    [128,1024] PSUM tile, double-buffered (ps_s bufs=2) so score J+1 never
    waits for exp J's PSUM read.  One strided exp covers both heads.
  - AV runs output-[q, hd]: lhsT is a 128-query window of the exp'd score
    tile pt (stationary), rhs is V plus a ones-column (65 moving cols per
    j-tile instead of up to 512) -- ~2x less PE time than the transposed
    orientation, and the softmax denominator rides in the ones column.  The
    8 per-window 65-col accumulation groups pack into two PSUM banks; only
    the first matmul into each bank carries start=True (bank-granular
    zeroing), the other groups self-initialize on still-pending bytes.
  - Each completed 128-query AV window is normalized per-partition
    (free [128,1] reciprocal + tensor_scalar muls on DVE), transposed back
    to [hd, q] by a cheap PE identity-matmul, and fed to the row-sharded
    projection, whose pair-0 half starts before the transpose lands.
  - One continuous software pipeline spans all eight (quarter, pair)
    attention passes: a global AV queue lags the score/exp stream by DELAY
    steps across att boundaries (TLAG for the tail att so its windows
    drain in-att), and all remaining PE work (V j-tiles, pair-1 QKV
    chunks, transposes, projection halves) is woven in as ~430ns fillers
    by a budgeted round-robin driver; completed quarters' projections are
    deferred toward the late, filler-poor, exp-paced atts (PDLY).
  - Emission-order guards force V/pair-1-QKV fillers out before any
    consumer is emitted, since tile-framework deps track only
    already-emitted instructions.
"""

from contextlib import ExitStack

import ml_dtypes
import numpy as np

B, S, D = 2, 2048, 1024
NH, HD = 16, 64
NCORES = 8
GROUPS = 4           # tensor-parallel head groups per batch
HPC = NH // GROUPS   # heads per core
SCALE = 1.0 / 8.0    # 1/sqrt(HD)

_CACHE = {}


def _body(ctx, tc, mybir, xt, wqk, wv, wp, qkb, vb, tri, ident, out):
    nc = tc.nc
    f32 = mybir.dt.float32
    f32r = mybir.dt.float32r
    bf16 = mybir.dt.bfloat16
    EXP = mybir.ActivationFunctionType.Exp
    COPY = mybir.ActivationFunctionType.Copy

    pin = ctx.enter_context(tc.tile_pool(name="pin", bufs=1))
    pwork = ctx.enter_context(tc.tile_pool(name="pwork", bufs=1))
    ppt = ctx.enter_context(tc.tile_pool(name="ppt", bufs=12))
    prec = ctx.enter_context(tc.tile_pool(name="prec", bufs=4))
    pat2 = ctx.enter_context(tc.tile_pool(name="pat2", bufs=8))
    pstage = ctx.enter_context(tc.tile_pool(name="pstage", bufs=8))
    ps_mm = ctx.enter_context(tc.tile_pool(name="ps_mm", bufs=2, space="PSUM"))
    ps_s = ctx.enter_context(tc.tile_pool(name="ps_s", bufs=2, space="PSUM"))
    ps_av = ctx.enter_context(tc.tile_pool(name="ps_av", bufs=1, space="PSUM"))

    # ---------------- input staging ----------------
    # small tensors first (qkb gates the first bias add, tri the first
    # diagonal mask); wqk/xt interleaved per k-tile feed phase A as they
    # land; wv next (V fillers); wp last (not needed until the projection).
    qkb_sb = pin.tile([128, 4], f32, name="qkb_sb")
    tri_sb = pin.tile([128, 256], bf16, name="tri_sb")
    ident_sb = pin.tile([128, 128], bf16, name="ident_sb")
    vb_sb = pin.tile([128, 256], f32, name="vb_sb")
    xt_sb = pin.tile([128, 8 * 2048], bf16, name="xt_sb")
    wqk_sb = pin.tile([128, 4096], bf16, name="wqk_sb")
    # sc-major xt stream: each transfer delivers one 512-column sequence
    # block across ALL k-tiles (strided dram->sbuf), so pair-0 Q/K for that
    # query/key range completes as soon as the block lands and attention
    # starts ~12us earlier than with a k-major stream.
    xt_v = xt.rearrange("(k p) c -> p k c", p=128)
    sb_v = xt_sb.rearrange("p (k c) -> p k c", c=2048)
    nc.sync.dma_start(wqk_sb[:, 0:2048], wqk[:, 0:2048])
    nc.sync.dma_start(sb_v[:, 0:4, 0:512], xt_v[:, 0:4, 0:512])
    nc.sync.dma_start(wqk_sb[:, 2048:4096], wqk[:, 2048:4096])
    nc.sync.dma_start(sb_v[:, 4:8, 0:512], xt_v[:, 4:8, 0:512])
    nc.sync.dma_start(qkb_sb[:], qkb[:])
    nc.sync.dma_start(tri_sb[:], tri[:])
    nc.sync.dma_start(ident_sb[:], ident[:])
    nc.sync.dma_start(vb_sb[:], vb[:])
    wv_sb = pin.tile([128, 2048], bf16, name="wv_sb")
    nc.sync.dma_start(wv_sb[:], wv[:])
    for sc in range(1, 4):
        nc.sync.dma_start(sb_v[:, :, sc * 512:(sc + 1) * 512],
                          xt_v[:, :, sc * 512:(sc + 1) * 512])
    wp_sb = pin.tile([128, 2048], bf16, name="wp_sb")

    # Q^T / K^T: head-pair p at cols [p*2048, (p+1)*2048); head hh of the pair
    # on partitions [hh*64, hh*64+64).
    qt_sb = pwork.tile([128, 2 * 2048], bf16, name="qt_sb")
    kt_sb = pwork.tile([128, 2 * 2048], bf16, name="kt_sb")
    # V: per j-tile 260 cols [V_h0 1 V_h1 1 V_h2 1 V_h3 1]: each head's AV
    # moving operand is the contiguous 65-col window [V_h | ones-col]; the
    # ones column emits the softmax denominator in the same matmul.
    v_sb = pwork.tile([128, 16 * 260], bf16, name="v_sb")
    # a^T: pair k2 at cols [k2*2048, ...), head hh on partitions hh*64..
    at_sb = pwork.tile([128, 2 * 2048], bf16, name="at_sb")

    # ones columns of v_sb, written once
    nc.vector.memset(
        v_sb.rearrange("p (g c) -> p g c", c=65)[:, :, 64:65], 1.0)
    nc.sync.dma_start(wp_sb[:], wp[:])

    # Dummy exp so the ACT table set loads during the input-DMA window instead
    # of delaying the first real softmax exp.
    warm = pin.tile([128, 4], f32, name="warm")
    nc.scalar.activation(warm[:], wqk_sb[:, 0:4], EXP, scale=0.0)

    # ---------------- QKV pair0: k-outer phase A ----------------
    # One matmul group per (Q/K, sc) with k outermost, so every arriving
    # x^T k-tile immediately feeds 8 x 512 cols of PE work: PE tracks the
    # input DMA stream instead of stalling on it.  The 8 accumulators borrow
    # all 8 PSUM banks (score slots hold two 512-wide groups side by side).
    sA = ps_s.tile([128, 1024], f32, tag="s", name="sA")
    for k in range(8):
        kw = dict(start=(k == 0), stop=(k == 7), skip_group_check=True)
        rhs = xt_sb[:, k * 2048: k * 2048 + 512]
        nc.tensor.matmul(sA[:, 0:512], lhsT=wqk_sb[:, k * 512: k * 512 + 128],
                         rhs=rhs, **kw)
        nc.tensor.matmul(sA[:, 512:1024],
                         lhsT=wqk_sb[:, k * 512 + 256: k * 512 + 384],
                         rhs=rhs, **kw)
    nc.vector.tensor_scalar_add(
        qt_sb[:, 0:512], sA[:, 0:512], qkb_sb[:, 0:1])
    nc.vector.tensor_scalar_add(
        kt_sb[:, 0:512], sA[:, 512:1024], qkb_sb[:, 2:3])

    # Remaining QKV (pair-0 sc1-3 and all of pair-1), split into ~427ns
    # filler chunks (2 k-tiles each), q fully before k so only ONE ps_mm
    # accumulator is ever open -- the other buffer stays free for
    # interleaved projection/transpose fillers.
    def qkv_fillers(Cs, scs):
        out_f = []
        for sc in scs:
            for C in Cs:
                state = {}

                def chunk(k2, sc=sc, C=C, state=state):
                    if k2 == 0:
                        state["t"] = ps_mm.tile(
                            [128, 512], f32, tag="acc", name=f"qk1_{sc}_{C}")
                    for k in (2 * k2, 2 * k2 + 1):
                        nc.tensor.matmul(
                            state["t"][:],
                            lhsT=wqk_sb[:, k * 512 + C * 128: k * 512 + C * 128 + 128],
                            rhs=xt_sb[:, k * 2048 + sc * 512: k * 2048 + (sc + 1) * 512],
                            start=(k == 0), stop=(k == 7))
                    if k2 == 3:
                        dst = qt_sb if C in (0, 1) else kt_sb
                        pair = C % 2
                        nc.vector.tensor_scalar_add(
                            dst[:, pair * 2048 + sc * 512: pair * 2048 + (sc + 1) * 512],
                            state["t"][:], qkb_sb[:, C:C + 1])
                        if C == 3:
                            qk1_done.add(sc)
                        if C == 2:
                            qk0_done.add(sc)

                for k2 in range(4):
                    out_f.append(lambda k2=k2, chunk=chunk: chunk(k2))
        return out_f

    def v_jtile_half(j, half, state):
        if half == 0:
            state["ps"] = ps_mm.tile([128, 256], f32, tag="acc", name="ps_v")
        ps = state["ps"]
        for k in range(half * 4, half * 4 + 4):
            nc.tensor.matmul(
                ps[:],
                lhsT=xt_sb[:, k * 2048 + j * 128: k * 2048 + (j + 1) * 128],
                rhs=wv_sb[:, k * 256:(k + 1) * 256],
                start=(k == 0), stop=(k == 7))
        if half == 1:
            nc.vector.tensor_add(
                v_sb[:, j * 260:(j + 1) * 260].rearrange(
                    "p (a c) -> p a c", c=65)[:, :, 0:64],
                ps.rearrange("p (a c) -> p a c", a=4),
                vb_sb.rearrange("p (a c) -> p a c", a=4))
            v_done.add(j)

    def v_fillers(lo, hi):
        out_f = []
        for j in range(lo, hi):
            state = {}
            out_f.append((430, lambda j=j, state=state: v_jtile_half(j, 0, state)))
            out_f.append((430, lambda j=j, state=state: v_jtile_half(j, 1, state)))
        return out_f

    # ---------------- attention ----------------
    # Processed per (i-quarter Q of 512, head-pair p).  Scores for both heads
    # of the pair share one [128, 1024] PSUM tile (head hh at cols hh*512), so
    # a single strided exp covers both.  AV accumulates output-[q, hd] per
    # 128-query window: 8 65-col groups (4 windows x 2 heads) packed into the
    # two ps_av banks; group (d, hh) sits at avA col 130*d + 65*hh for
    # d < 3 or hh == 0, and (3, 1) alone in avB.

    def proj_half(st, ec, on_act=False):
        ps = ps_mm.tile([128, 512], f32, tag="acc", name="ps_o")
        for k2 in range(2):
            nc.tensor.matmul(
                ps[:],
                lhsT=at_sb[:, k2 * 2048 + st * 128: k2 * 2048 + (st + 1) * 128],
                rhs=wp_sb[:, k2 * 1024 + ec * 512: k2 * 1024 + (ec + 1) * 512],
                start=(k2 == 0), stop=(k2 == 1))
        stage = pstage.tile([128, 512], bf16, tag="stage", name="stage")
        if on_act:
            # tail path: ACT is idle after the last exp; keep DVE free
            nc.scalar.activation(stage[:], ps[:], COPY)
        else:
            nc.vector.tensor_copy(stage[:], ps[:])
        nc.sync.dma_start(
            out[st * 128:(st + 1) * 128, ec * 512:(ec + 1) * 512], stage[:])

    DELAY = 5  # software-pipeline distance between scores/exp and AV use
    TLAG = 3   # shorter AV lag for the tail att
    BUD = 120  # per-step filler budget pad over the ACT-pace estimate
    PDLY = (28, 22, 14, 1)  # per-quarter projection deferral (steps)

    # two-level filler queues: hi carries dependency-bearing work (V j-tiles,
    # pair-1 QKV); lo carries transposes/projections.  Driver pops hi first.
    # av_q carries deferred AV step closures: the AV stream lags the GLOBAL
    # score stream by DELAY steps, crossing att boundaries, so no att ever
    # idles through a drain phase.  deferred holds window-completion work
    # (transpose/proj) flushed one step later so it never head-blocks.
    fill_hi = []
    fill_lo = []
    av_q = []
    deferred = []  # [steps_to_wait, cost, fn]; flushed to fill_lo at 0
    v_done = set()     # j-tiles whose V columns have been emitted
    qk1_done = set()   # pair-1 sc blocks fully emitted
    qk0_done = {0}     # pair-0 sc blocks fully emitted (sc0 in phase A)

    def att_qp(Q, p, tail=False):
        """Generator emitting scores/exp per J iteration and queueing the
        matching AV step on av_q; yields the score width w so the driver can
        weave PE filler chunks into the ACT-paced stretches.  AV windows
        complete per 128-query tile; each completion emits recip +
        per-partition normalize (DVE) inline, and defers the PE transpose
        (+ projection for p==1) to the lo filler queue."""
        qlo = Q * 512
        nJ = 4 * Q + 4
        avA = ps_av.tile([128, 512], f32, tag="ava", name="avA")
        avB = ps_av.tile([128, 512], f32, tag="avb", name="avB")
        rec = prec.tile([128, 8], f32, tag="rec", name="rec")

        def av_ap(d, hh):
            if d == 3 and hh == 1:
                return avB[:, 0:65]
            off = 130 * d + 65 * hh
            return avA[:, off:off + 65]

        def window_done(d):
            # softmax denominators for window d sit in col 64 of each group
            if d < 3:
                nc.vector.reciprocal(
                    rec[:, 2 * d:2 * d + 2],
                    avA[:, 130 * d:130 * d + 130].rearrange(
                        "p (g c) -> p g c", c=65)[:, :, 64])
            else:
                nc.vector.reciprocal(rec[:, 6:7], avA[:, 454:455])
                nc.vector.reciprocal(rec[:, 7:8], avB[:, 64:65])
            at2 = pat2.tile([128, 128], bf16, tag="at2", name="at2")
            # h0 on DVE (the Pool TensorScalarPtr rejects f32-psum-in ->
            # bf16-out in the BIR verifier); in the tail h1 runs as an ACT
            # scaled-copy in parallel since ACT is idle there
            nc.vector.tensor_scalar_mul(
                at2[:, 0:64], av_ap(d, 0)[:, 0:64], rec[:, 2 * d:2 * d + 1])
            if tail:
                nc.scalar.activation(
                    at2[:, 64:128], av_ap(d, 1)[:, 0:64], COPY,
                    scale=rec[:, 2 * d + 1:2 * d + 2])
            else:
                nc.vector.tensor_scalar_mul(
                    at2[:, 64:128], av_ap(d, 1)[:, 0:64], rec[:, 2 * d + 1:2 * d + 2])
            t = 4 * Q + d

            def fin(at2=at2, t=t):
                tp = ps_mm.tile([128, 128], bf16, tag="acc", name="tp")
                nc.tensor.transpose(tp[:], at2[:], ident_sb[:])
                nc.vector.tensor_copy(
                    at_sb[:, p * 2048 + t * 128: p * 2048 + (t + 1) * 128], tp[:])

            if p == 0:
                deferred.append([1, 60, fin])
            else:
                oa = tail
                # push completed quarters' projections toward the late,
                # filler-poor atts: they have no downstream consumers
                dly = PDLY[Q]
                st = {}

                # each ec's projection is split in two adjacent fillers: A
                # runs the pair-0 contraction half (no dependency on this
                # window's at_sb copy); B, popped a step later, runs the
                # pair-1 half once the copy has safely landed
                def prA(ec, t=t, st=st):
                    ps = st[ec] = ps_mm.tile([128, 512], f32, tag="acc", name="ps_o")
                    nc.tensor.matmul(
                        ps[:], lhsT=at_sb[:, t * 128:(t + 1) * 128],
                        rhs=wp_sb[:, ec * 512:(ec + 1) * 512], start=True, stop=False)

                def prB(ec, t=t, st=st, oa=oa):
                    ps = st[ec]
                    nc.tensor.matmul(
                        ps[:], lhsT=at_sb[:, 2048 + t * 128: 2048 + (t + 1) * 128],
                        rhs=wp_sb[:, 1024 + ec * 512: 1024 + (ec + 1) * 512],
                        start=False, stop=True)
                    stage = pstage.tile([128, 512], bf16, tag="stage", name="stage")
                    if oa:
                        # tail: stage on ACT, DMA via the ACT hwdge queue so
                        # the final descriptors don't serialize behind SP's
                        nc.scalar.activation(stage[:], ps[:], COPY)
                        nc.scalar.dma_start(
                            out[t * 128:(t + 1) * 128, ec * 512:(ec + 1) * 512], stage[:])
                    else:
                        nc.vector.tensor_copy(stage[:], ps[:])
                        nc.sync.dma_start(
                            out[t * 128:(t + 1) * 128, ec * 512:(ec + 1) * 512], stage[:])

                deferred.append([1, 60, fin])
                deferred.append([dly, 270, lambda: prA(0)])
                deferred.append([dly + 1, 210, lambda: prB(0)])
                deferred.append([dly + 1, 210, lambda: prA(1)])
                deferred.append([dly + 2, 210, lambda: prB(1)])

        for J in range(nJ):
            jlo = J * 128
            istart = max(jlo, qlo)
            w = qlo + 512 - istart
            pss = ps_s.tile([128, 1024], f32, tag="s", name="pss")
            for hh in range(2):
                nc.tensor.matmul(
                    pss[:, hh * 512: hh * 512 + w],
                    lhsT=kt_sb[hh * 64:(hh + 1) * 64, p * 2048 + jlo: p * 2048 + jlo + 128],
                    rhs=qt_sb[hh * 64:(hh + 1) * 64, p * 2048 + istart: p * 2048 + istart + w],
                    start=True, stop=True)
            pt = ppt.tile([128, 1024], bf16, tag="pt", name="pt")
            nc.scalar.activation(
                pt.rearrange("x (h c) -> x h c", c=512)[:, :, 0:w],
                pss.rearrange("x (h c) -> x h c", c=512)[:, :, 0:w],
                EXP, scale=SCALE)
            if jlo >= qlo:
                # diagonal j-tile: zero the j > i triangle of both heads in
                # one strided op (tri_sb holds the mask twice); on DVE the
                # all-bf16 all-sbuf op hits the fast perf mode
                dia = pt.rearrange("x (h c) -> x h c", c=512)[:, :, 0:128]
                nc.vector.tensor_mul(
                    dia, dia, tri_sb.rearrange("x (h c) -> x h c", c=128))

            def av_step(Ja=J, pt=pt, cod=(istart - qlo) // 128):
                d0 = max(0, Ja - 4 * Q)
                for hh in range(2):
                    for d in range(d0, 4):
                        # start=True zeroes the WHOLE 2KB psum bank
                        # (zero-region granularity), so only the first matmul
                        # into each av bank may carry it; sibling groups'
                        # first writes hit pending bytes and self-initialize.
                        first_of_bank = Ja == 0 and (
                            (hh == 0 and d == 0) or (hh == 1 and d == 3))
                        nc.tensor.matmul(
                            av_ap(d, hh),
                            lhsT=pt[:, hh * 512 + 128 * (d - cod): hh * 512 + 128 * (d - cod) + 128],
                            rhs=v_sb[:, Ja * 260 + p * 130 + hh * 65: Ja * 260 + p * 130 + hh * 65 + 65],
                            start=first_of_bank, stop=(Ja == 4 * Q + d),
                            skip_group_check=True)
                if Ja - 4 * Q >= 0:
                    window_done(Ja - 4 * Q)

            av_step.vj = J
            av_q.append(av_step)
            yield w

    # ---------------- driver: one continuous pipeline ----------------
    def flush_deferred():
        keep = []
        for item in deferred:
            item[0] -= 1
            if item[0] <= 0:
                fill_lo.append((item[1], item[2]))
            else:
                keep.append(item)
        deferred[:] = keep

    def step_tail(budget, lag=DELAY):
        flush_deferred()
        while len(av_q) > lag:
            st = av_q[0]
            # emission-order guard: the V columns an AV step reads must be
            # emitted before the AV matmuls, else no dependency is recorded
            while st.vj not in v_done and fill_hi:
                fill_hi.pop(0)[1]()
            av_q.pop(0)()
        # round-robin hi/lo so neither dependency-bearing fillers nor the
        # projection stream can starve the other
        take_hi = True
        while budget > 0 and (fill_hi or fill_lo):
            src = fill_hi if (take_hi and fill_hi) or not fill_lo else fill_lo
            cost, fn = src.pop(0)
            fn()
            budget -= cost
            take_hi = not take_hi

    gstep = [0]

    def drive(gen, need_sc=None, need_sc0=None, lag=DELAY, nJ=None):
        if need_sc is not None:
            # emission-order guard for pair-1 qt/kt reads by this att's scores
            while not all(s in qk1_done for s in range(need_sc + 1)) and fill_hi:
                fill_hi.pop(0)[1]()
        if need_sc0 is not None:
            while not all(s in qk0_done for s in range(need_sc0 + 1)) and fill_hi:
                fill_hi.pop(0)[1]()
        for J, w in enumerate(gen):
            # pop just enough to cover this step's ACT pace + latency slack,
            # preserving filler backlog for the filler-poor late quarters;
            # the first latency-paced atts absorb more
            gstep[0] += 1
            step = lag
            if nJ is not None:
                # tail att: shrink the AV lag over the last steps so the
                # window-drain chains overlap the still-running exp stream
                step = max(1, min(lag, nJ - 2 - J))
            step_tail(int(0.833 * w) + BUD, lag=step)

    qk0 = qkv_fillers((0, 2), (1, 2, 3))  # pair-0 sc1-3: 8 chunks per sc
    qk1 = qkv_fillers((1, 3), (0, 1, 2, 3))  # pair-1: [8*sc : 8*sc+8]
    # static fillers seeded just-in-time: early enough that the emission
    # guards never fire, late enough that the filler-poor late quarters
    # still have mass to absorb PE slack
    fill_hi.extend(v_fillers(0, 4))
    fill_hi.extend((430, f) for f in qk1[0:8])
    fill_hi.extend((430, f) for f in qk0[0:8])
    drive(att_qp(0, 0))
    fill_hi.extend(v_fillers(4, 8))
    fill_hi.extend((430, f) for f in qk0[8:16])
    drive(att_qp(1, 0), need_sc0=1)
    fill_hi.extend((430, f) for f in qk1[8:16])
    drive(att_qp(0, 1), need_sc=0)
    fill_hi.extend(v_fillers(8, 12))
    fill_hi.extend((430, f) for f in qk0[16:24])
    drive(att_qp(1, 1), need_sc=1)
    fill_hi.extend((430, f) for f in qk1[16:24])
    drive(att_qp(2, 0), need_sc0=2)
    fill_hi.extend(v_fillers(12, 16))
    drive(att_qp(2, 1), need_sc=2)
    fill_hi.extend((430, f) for f in qk1[24:32])
    drive(att_qp(3, 0), need_sc0=3)
    # the tail att runs with a short AV lag so its windows complete (and
    # their projections flow) while its own exp stream still paces the PE
    drive(att_qp(3, 1, tail=True), need_sc=3, lag=DELAY, nJ=16)
    # epilogue: alternate the remaining AV steps (the critical tail chain)
    # with deferred flushes and filler pops; once the AVs are drained, pop
    # everything that's ready each round
    while av_q or deferred or fill_hi or fill_lo:
        flush_deferred()
        if av_q:
            av_q.pop(0)()
            if fill_hi or fill_lo:
                cost, fn = (fill_hi or fill_lo).pop(0) if fill_hi else fill_lo.pop(0)
                fn()
        else:
            while fill_hi or fill_lo:
                cost, fn = (fill_hi or fill_lo).pop(0) if fill_hi else fill_lo.pop(0)
                fn()


def _build_nc(repeat=1):
    key = ("nc", repeat)
    if key in _CACHE:
        return _CACHE[key]
    import concourse.bacc as bacc
    import concourse.mybir as mybir
    import concourse.tile as tile

    f32 = mybir.dt.float32
    f32r = mybir.dt.float32r
    bf16d = mybir.dt.bfloat16
    nc = bacc.Bacc("TRN2", target_bir_lowering=False, debug=False)
    xt = nc.dram_tensor("xt", [D, S], bf16d, kind="ExternalInput").ap()
    wqk = nc.dram_tensor("wqk", [128, 4096], bf16d, kind="ExternalInput").ap()
    wv = nc.dram_tensor("wv", [128, 2048], bf16d, kind="ExternalInput").ap()
    wp = nc.dram_tensor("wp", [128, 2048], bf16d, kind="ExternalInput").ap()
    qkb = nc.dram_tensor("qkb", [128, 4], f32, kind="ExternalInput").ap()
    vb = nc.dram_tensor("vb", [128, 256], f32, kind="ExternalInput").ap()
    tri = nc.dram_tensor("tri", [128, 256], mybir.dt.bfloat16, kind="ExternalInput").ap()
    ident = nc.dram_tensor("ident", [128, 128], mybir.dt.bfloat16, kind="ExternalInput").ap()
    out = nc.dram_tensor("out", [S, D], bf16d, kind="ExternalOutput").ap()

    with tile.TileContext(nc) as tc:
        for _ in range(repeat):
            with ExitStack() as ctx:
                _body(ctx, tc, mybir, xt, wqk, wv, wp, qkb, vb, tri, ident, out)
    nc.compile()
    _CACHE[key] = nc
    return nc


def _make_in_maps(hidden_states, c_attn_w, c_attn_b, c_proj_w):
    hs = np.asarray(hidden_states, dtype=np.float32)
    waw = np.asarray(c_attn_w, dtype=np.float32)
    wab = np.asarray(c_attn_b, dtype=np.float32)
    wpw = np.asarray(c_proj_w, dtype=np.float32)

    tri = np.tile(np.triu(np.ones((128, 128), dtype=ml_dtypes.bfloat16)), 2)
    ident_host = np.eye(128, dtype=ml_dtypes.bfloat16)
    xts = [np.ascontiguousarray(hs[b].T).astype(ml_dtypes.bfloat16) for b in range(B)]
    in_maps = []
    for c in range(NCORES):
        b, g = divmod(c, GROUPS)
        cols = np.arange(g * HPC * HD, (g + 1) * HPC * HD)
        wqk_host = np.concatenate([waw[:, cols], waw[:, D + cols]], axis=1)
        in_maps.append({
            "xt": xts[b],
            "wqk": np.ascontiguousarray(
                wqk_host.reshape(8, 128, 512).transpose(1, 0, 2).reshape(128, 4096)).astype(ml_dtypes.bfloat16),
            "wv": np.ascontiguousarray(
                waw[:, 2 * D + cols].reshape(8, 128, 256).transpose(1, 0, 2).reshape(128, 2048)).astype(ml_dtypes.bfloat16),
            "wp": np.ascontiguousarray(
                wpw[cols, :].reshape(2, 128, 1024).transpose(1, 0, 2).reshape(128, 2048)).astype(ml_dtypes.bfloat16),
            "qkb": np.ascontiguousarray(
                np.concatenate([wab[cols], wab[D + cols]]).reshape(4, 128).T),
            "vb": np.ascontiguousarray(
                np.broadcast_to(wab[2 * D + cols], (128, 256))),
            "tri": tri,
            "ident": ident_host,
        })
    return in_maps


def kernel(hidden_states, c_attn_w, c_attn_b, c_proj_w, c_proj_b):
    from concourse import bass_utils

    nc = _build_nc()
    in_maps = _make_in_maps(hidden_states, c_attn_w, c_attn_b, c_proj_w)
    res = bass_utils.run_bass_kernel_spmd(nc, in_maps, core_ids=list(range(NCORES)))
    outs = [np.asarray(r["out"], dtype=np.float32) for r in res.results]
    wpb = np.asarray(c_proj_b, dtype=np.float32)
    full = np.stack(
        [sum(outs[b * GROUPS:(b + 1) * GROUPS]) + wpb for b in range(B)], axis=0)
    return full.astype(np.float32)
